# revision 20
# baseline (speedup 1.0000x reference)
"""Trainium2 Bass kernel for nn_Cross_MultiAttention (8-head cross attention).

Sharding: one attention head per NeuronCore (8 heads / 8 cores).

Host folds the shared 1x1 input conv into each head's q/k/v projections
(Aq = wq_h @ w_in etc.), so each core:
  - projects q/k/v for its head directly from (x+pos) / (context+pos), f16,
  - computes the full 5000x5000 attention for its head with scores kept
    TRANSPOSED (keys on partitions, queries on the free dim). Softmax is
    max-free (|scores/16| < ~4); the denominator comes from an appended
    ones-column in V.
  - P@V runs 2-wide column-tiled on the PE array (128x64 mode): two key
    tiles stream concurrently, each with a [V | ones] 33-col stationary.
    PV matmul pairs are interleaved into the QK loop of the next stripe so
    the ScalarE exp stream (the bottleneck) never starves.
  - keys are host-padded to 5120 (mask=0 on pad keys) so all 40 key tiles
    are full 128 partitions.
  - mask is f16 (DVE 2x 16-bit perf mode) and host-swizzled so each
    partition's bytes are DRAM-contiguous per stripe: DMA runs in 8-keytile
    chunks at ~16KB/descriptor instead of 2KB rows (the packet rate, not
    bandwidth, is the DMA ceiling).
  - epilogue folds the cross-column-group reduction into the output
    projection stationary (rows 0-31 and 64-95 both carry w_out slice).
Host divides each partial [256, 5000] by its denominator row, sums the 8
partials, adds b_out, reshapes to [256, 50, 100].
"""

import numpy as np

import concourse.bacc as bacc
import concourse.tile as tile
import concourse.mybir as mybir
from concourse.bass_utils import run_bass_kernel_spmd

F32 = mybir.dt.float32
F32R = mybir.dt.float32r  # fp32 bits, full-rate PE streaming mode (rounded)
F16 = mybir.dt.float16
AF = mybir.ActivationFunctionType

EMB = 256
HEADS = 8
DEPTH = 32
IN_CH = 256
H, W = 50, 100
N_TOK = H * W          # 5000 queries
N_KEY = 5120           # keys padded to a multiple of 128
NQP = 5120             # queries padded (mask layout only)
SCALE = EMB ** (-0.5)  # 1/16
WSZ = 1024             # query stripe width
NJ = N_KEY // 128      # 40 key tiles
NW = 5                 # query stripes
CH = 8                 # key tiles per mask DMA chunk
D = DEPTH
QK_F16PSUM = False     # f16 PSUM matmul output rejected by bass
SCHRAU_MOD = 0         # disabled: DVE 1x cost on f32 PSUM src outweighs gain
GP_MOD = 0             # disabled: GpSimd mul too slow + SBUF port contention
# Schraudolph f16 exp: bits = s * (1024/(64*ln2)) + (15*1024 - 44.6 + 0.5)
SCH_A = 1024.0 / (64.0 * 0.6931471805599453)
SCH_B = 15315.9


def _tiles(total, size):
    out = []
    p = 0
    while p < total:
        out.append((p, min(size, total - p)))
        p += size
    return out


def build_nc(num_devices=8):
    nc = bacc.Bacc("TRN2", target_bir_lowering=False, debug=False,
                   num_devices=num_devices)

    # packed inputs: per-partition bytes contiguous in DRAM
    xp_d = nc.dram_tensor("xp", (128, NW, 2, WSZ), F16, kind="ExternalInput").ap()
    cp_d = nc.dram_tensor("cp", (128, NW, 2, WSZ), F16, kind="ExternalInput").ap()
    nmT_d = nc.dram_tensor("nmT", (128, NW, NJ, WSZ), F16, kind="ExternalInput").ap()
    AqT_d = nc.dram_tensor("AqT", (IN_CH, 4 * D), F16, kind="ExternalInput").ap()
    cq_d = nc.dram_tensor("cq", (4 * D, 1), F32, kind="ExternalInput").ap()
    AkT_d = nc.dram_tensor("AkT", (IN_CH, 4 * D), F16, kind="ExternalInput").ap()
    ck_d = nc.dram_tensor("ck", (4 * D, 1), F32, kind="ExternalInput").ap()
    AvT_d = nc.dram_tensor("AvT", (IN_CH, D), F16, kind="ExternalInput").ap()
    cvb_d = nc.dram_tensor("cvb", (128, D), F32, kind="ExternalInput").ap()
    ly_d = nc.dram_tensor("ly", (128, EMB), F32R, kind="ExternalInput").ap()
    sdn_d = nc.dram_tensor("sdn", (128, 128), F32R, kind="ExternalInput").ap()
    y_d = nc.dram_tensor("y", (EMB, N_TOK), F32, kind="ExternalOutput").ap()
    dn_d = nc.dram_tensor("dn", (1, N_TOK), F32, kind="ExternalOutput").ap()

    wtiles = _tiles(N_TOK, WSZ)   # query stripes (last = 904)

    with tile.TileContext(nc) as tc:
        with (
            tc.tile_pool(name="persist", bufs=1) as persist,
            tc.tile_pool(name="consts", bufs=1) as consts,
            tc.tile_pool(name="s_ps", bufs=2, space="PSUM") as s_ps,
            tc.tile_pool(name="av_ps", bufs=2, space="PSUM") as av_ps,
            tc.tile_pool(name="proj_in", bufs=3) as proj_in,
            tc.tile_pool(name="m_sb", bufs=3) as m_pool,
            tc.tile_pool(name="out_sb", bufs=3) as out_pool,
        ):
            # ---- constants to SBUF ----
            AqT_sb = consts.tile([128, 2, 4 * D], F16)
            AkT_sb = consts.tile([128, 2, 4 * D], F16)
            AvT_sb = consts.tile([128, 2, D], F16)
            for ct in range(2):
                nc.sync.dma_start(AqT_sb[:, ct, :], AqT_d[ct * 128:(ct + 1) * 128, :])
                nc.sync.dma_start(AkT_sb[:, ct, :], AkT_d[ct * 128:(ct + 1) * 128, :])
                nc.sync.dma_start(AvT_sb[:, ct, :], AvT_d[ct * 128:(ct + 1) * 128, :])
            cq_sb = consts.tile([4 * D, 1], F32)
            nc.sync.dma_start(cq_sb[:, :], cq_d[:, :])
            ck_sb = consts.tile([4 * D, 1], F32)
            nc.sync.dma_start(ck_sb[:, :], ck_d[:, :])
            cvb_sb = consts.tile([128, D], F32)
            nc.sync.dma_start(cvb_sb[:, :], cvb_d[:, :])
            ly_sb = consts.tile([128, EMB], F32R)
            nc.sync.dma_start(ly_sb[:, :], ly_d[:, :])
            sdn_sb = consts.tile([128, 128], F32R)
            nc.sync.dma_start(sdn_sb[:, :], sdn_d[:, :])

            # warm up the exp table load early (hides ~2.7us)
            dumm = consts.tile([128, 16], F32)
            nc.any.memset(dumm[:, :], 0.0)
            dummo = consts.tile([128, 16], F32)
            nc.scalar.activation(dummo[:, :], dumm[:, :], AF.Exp, scale=1.0)

            # ---- persistent activations ----
            qT = persist.tile([4 * D, N_TOK], F16)
            kT = persist.tile([4 * D, N_KEY], F16)
            v_sb = persist.tile([128, NJ, D + 1], F16)  # [j % 128, jt, d | ones]
            ones_stage = consts.tile([128, NJ], F32)
            nc.any.memset(ones_stage[:, :], 1.0)
            nc.vector.tensor_copy(v_sb[:, :, D], ones_stage[:, :])
            # probability stripe: all NJ key-tiles for one query stripe
            p_store = persist.tile([128, NJ, WSZ], F16)

            # zero the av PSUM banks once so never-written rows stay finite
            for _ in range(2):
                za = av_ps.tile([128, 512], F32, name="za", tag="ava")
                zb = av_ps.tile([128, 512], F32, name="zb", tag="avb")
                nc.vector.memset(za[:, :], 0.0)
                nc.vector.memset(zb[:, :], 0.0)

            # ---- stage 1: project q/k/v straight from (x|context)+pos ----
            def proj_q(n):
                n0, ns = wtiles[n]
                img_t = proj_in.tile([128, 2, WSZ], F16, name="img_t")
                nc.sync.dma_start(img_t[:, :, :], xp_d[:, n, :, :])
                qps = s_ps.tile([128, WSZ], F32, name="qps", tag="s")
                for ct in range(2):
                    for (h0, hs) in _tiles(ns, 512):
                        nc.tensor.matmul(qps[:, h0:h0 + hs], AqT_sb[:, ct, :],
                                         img_t[:, ct, h0:h0 + hs],
                                         start=(ct == 0), stop=(ct == 1))
                nc.vector.tensor_scalar_add(qT[:, n0:n0 + ns], qps[:, :ns],
                                            cq_sb[:, :])

            def proj_kv(n):
                n0 = n * WSZ
                img_t = proj_in.tile([128, 2, WSZ], F16, name="img_t")
                nc.sync.dma_start(img_t[:, :, :], cp_d[:, n, :, :])
                kps = s_ps.tile([128, WSZ], F32, name="kps", tag="s")
                for ct in range(2):
                    for h0 in (0, 512):
                        nc.tensor.matmul(kps[:, h0:h0 + 512], AkT_sb[:, ct, :],
                                         img_t[:, ct, h0:h0 + 512],
                                         start=(ct == 0), stop=(ct == 1))
                nc.vector.tensor_scalar_add(kT[:, n0:n0 + WSZ], kps[:, :],
                                            ck_sb[:, :])
                # v projection for the j-tiles inside this stripe
                for jj0 in range(0, WSZ, 128):
                    jt = (n0 + jj0) // 128
                    vps = av_ps.tile([128, 512], F32, name="vps", tag="ava")
                    for ct in range(2):
                        nc.tensor.matmul(vps[:, 0:D],
                                         img_t[:, ct, jj0:jj0 + 128],
                                         AvT_sb[:, ct, :],
                                         start=(ct == 0), stop=(ct == 1))
                    nc.vector.tensor_add(v_sb[:, jt, 0:D], vps[:, 0:D],
                                         cvb_sb[:, :])

            # stripe-0 queries, then all keys; remaining queries are issued
            # inside the w=0 attention loop (spread over jt steps)
            proj_q(0)
            for n in range(NW):
                proj_kv(n)

            # ---- stage 2: pipelined attention + output projection ----
            pending_epi = []
            for w in range(NW + 1):
                avab = None
                if w >= 1:
                    i0p, iszp = wtiles[w - 1]
                    phalf = _tiles(iszp, 512)
                    # one av pair per 512-half; both halves live concurrently
                    avab = [(av_ps.tile([128, 512], F32, name="av_a", tag="ava"),
                             av_ps.tile([128, 512], F32, name="av_b", tag="avb"))
                            for _ in phalf]
                i0, isz = wtiles[w] if w < NW else (0, 0)

                def load_chunk(c):
                    mchk = m_pool.tile([128, CH, WSZ], F16, name="mchk")
                    nc.sync.dma_start(mchk[:, :, :],
                                      nmT_d[:, w, c * CH:(c + 1) * CH, :])
                    return mchk

                cur_chk = nxt_chk = None
                for jt in range(NJ):
                    if w < NW:
                        # mask chunk prefetch, one chunk ahead
                        if jt == 0:
                            cur_chk = load_chunk(0)
                            nxt_chk = load_chunk(1)
                        elif jt % CH == 0:
                            cur_chk = nxt_chk
                            if (jt // CH + 1) * CH < NJ:
                                nxt_chk = load_chunk(jt // CH + 1)
                    # PV pairs for the previous stripe, interleaved two at a
                    # time (fewer PE mode switches); must be issued BEFORE
                    # this jt's exp overwrites p_store[jt]
                    if w >= 1 and jt % 4 == 0:
                        np2 = NJ // 2
                        for jq in (jt, jt + 2):
                            jp = jq // 2
                            for hi, (h0, hs) in enumerate(phalf):
                                av_a, av_b = avab[hi]
                                nc.tensor.matmul(
                                    av_a[0:D + 1, :hs], v_sb[:, jq, :],
                                    p_store[:, jq, h0:h0 + hs],
                                    start=(jp == 0), stop=(jp == np2 - 1),
                                    tile_position=(0, 0), skip_group_check=True)
                                nc.tensor.matmul(
                                    av_b[64:64 + D + 1, :hs], v_sb[:, jq + 1, :],
                                    p_store[:, jq + 1, h0:h0 + hs],
                                    start=(jp == 0), stop=(jp == np2 - 1),
                                    tile_position=(0, 64), skip_group_check=True)
                    # deferred epilogue matmuls for stripe w-2: av2sb copies
                    # happened last iteration, so these never head-of-line
                    # block the exp-feeding QK stream at a stripe boundary
                    if jt == 2 and pending_epi:
                        for (e_i0, e_h0, e_hs, e_av2) in pending_epi:
                            for c2 in range(2):
                                yps = s_ps.tile([128, WSZ], F32, name="yps",
                                                tag="s")
                                nc.tensor.matmul(
                                    yps[:, :e_hs],
                                    ly_sb[:, c2 * 128:(c2 + 1) * 128],
                                    e_av2[:, :e_hs], start=True, stop=True)
                                ysb = out_pool.tile([128, 512], F32, name="ysb")
                                nc.vector.tensor_copy(ysb[:, :e_hs],
                                                      yps[:, :e_hs])
                                nc.sync.dma_start(
                                    y_d[c2 * 128:(c2 + 1) * 128,
                                        e_i0 + e_h0:e_i0 + e_h0 + e_hs],
                                    ysb[:, :e_hs])
                            dps = s_ps.tile([128, WSZ], F32, name="dps",
                                            tag="s")
                            nc.tensor.matmul(dps[:, :e_hs], sdn_sb[:, :],
                                             e_av2[:, :e_hs],
                                             start=True, stop=True)
                            dnsb = out_pool.tile([1, 512], F32, name="dnsb")
                            nc.vector.tensor_copy(dnsb[:, :e_hs],
                                                  dps[32:33, :e_hs])
                            nc.sync.dma_start(
                                dn_d[:, e_i0 + e_h0:e_i0 + e_h0 + e_hs],
                                dnsb[:, :e_hs])
                        pending_epi = []
                    if w < NW:
                        j0 = jt * 128
                        if QK_F16PSUM:
                            s = s_ps.tile([128, WSZ], F16, name="s", tag="s")
                            nc.tensor.matmul(
                                s[:, :isz], kT[:, j0:j0 + 128],
                                qT[:, i0:i0 + isz], start=True, stop=True)
                        else:
                            s = s_ps.tile([128, WSZ], F32, name="s", tag="s")
                            for (h0, hs) in _tiles(isz, 512):
                                nc.tensor.matmul(
                                    s[:, h0:h0 + hs], kT[:, j0:j0 + 128],
                                    qT[:, i0 + h0:i0 + h0 + hs],
                                    start=True, stop=True)
                        if SCHRAU_MOD and jt % SCHRAU_MOD == 1:
                            # exp via f16 bit trick on DVE (max rel err ~3%)
                            nc.vector.tensor_scalar(
                                p_store[:, jt, :isz].bitcast(mybir.dt.int16),
                                s[:, :isz], SCH_A, SCH_B,
                                mybir.AluOpType.mult, mybir.AluOpType.add)
                        else:
                            nc.scalar.activation(p_store[:, jt, :isz],
                                                 s[:, :isz],
                                                 AF.Exp, scale=float(SCALE) / 4.0)
                        mul_eng = (nc.gpsimd if (GP_MOD and jt % GP_MOD == 0)
                                   else nc.vector)
                        mul_eng.tensor_mul(p_store[:, jt, :isz],
                                           p_store[:, jt, :isz],
                                           cur_chk[:, jt % CH, :isz])
                    # remaining query projections, spread through stripe 0
                    if w == 0 and jt >= 10 and jt < 10 + 2 * (NW - 1) and jt % 2 == 0:
                        proj_q((jt - 10) // 2 + 1)
                # stripe w-1: drain av PSUM to SBUF now (frees the av banks
                # for the next stripe); defer the projection matmuls
                if w >= 1:
                    for hi, (h0, hs) in enumerate(phalf):
                        av_a, av_b = avab[hi]
                        av2sb = out_pool.tile([128, 512], F32R, name="av2sb",
                                              tag="av2", bufs=4)
                        nc.vector.tensor_copy(av2sb[0:64, :hs], av_a[0:64, :hs])
                        nc.vector.tensor_copy(av2sb[64:128, :hs],
                                              av_b[64:128, :hs])
                        pending_epi.append((i0p, h0, hs, av2sb))

            # flush the last stripe's deferred epilogue
            for (e_i0, e_h0, e_hs, e_av2) in pending_epi:
                for c2 in range(2):
                    yps = s_ps.tile([128, WSZ], F32, name="yps", tag="s")
                    nc.tensor.matmul(yps[:, :e_hs],
                                     ly_sb[:, c2 * 128:(c2 + 1) * 128],
                                     e_av2[:, :e_hs], start=True, stop=True)
                    ysb = out_pool.tile([128, 512], F32, name="ysb")
                    nc.vector.tensor_copy(ysb[:, :e_hs], yps[:, :e_hs])
                    nc.sync.dma_start(
                        y_d[c2 * 128:(c2 + 1) * 128,
                            e_i0 + e_h0:e_i0 + e_h0 + e_hs],
                        ysb[:, :e_hs])
                dps = s_ps.tile([128, WSZ], F32, name="dps", tag="s")
                nc.tensor.matmul(dps[:, :e_hs], sdn_sb[:, :], e_av2[:, :e_hs],
                                 start=True, stop=True)
                dnsb = out_pool.tile([1, 512], F32, name="dnsb")
                nc.vector.tensor_copy(dnsb[:, :e_hs], dps[32:33, :e_hs])
                nc.sync.dma_start(dn_d[:, e_i0 + e_h0:e_i0 + e_h0 + e_hs],
                                  dnsb[:, :e_hs])

    nc.compile()
    return nc


def make_pos(row_embed, col_embed):
    """[EMB, H*W]; first half col embeds, second half row embeds."""
    d2 = row_embed.shape[1]
    pos = np.empty((EMB, H, W), np.float32)
    pos[:d2] = col_embed[:W].T[:, None, :]      # [d2, 1, W] -> broadcast H
    pos[d2:] = row_embed[:H].T[:, :, None]      # [d2, H, 1] -> broadcast W
    return pos.reshape(EMB, H * W)


def _pack_img(a):
    # [256, ncol] f32 -> [128, ntile, 2, WSZ] f16, per-partition contiguous
    ncol = a.shape[1]
    nt = (ncol + WSZ - 1) // WSZ
    out = np.zeros((2, 128, nt, WSZ), np.float16)
    a16 = a.astype(np.float16)
    for ct in range(2):
        flat = np.zeros((128, nt * WSZ), np.float16)
        flat[:, :ncol] = a16[ct * 128:(ct + 1) * 128]
        out[ct] = flat.reshape(128, nt, WSZ)
    return np.ascontiguousarray(out.transpose(1, 2, 0, 3))


def make_in_maps(x, context, pad_mask, row_embed, col_embed, w_in, b_in,
                 wq, bq, wk, bk, wv, bv, w_out, n_heads=HEADS):
    f8 = np.float64
    x = np.asarray(x, np.float32)
    context = np.asarray(context, np.float32)
    pad_mask = np.asarray(pad_mask)
    row_embed = np.asarray(row_embed, np.float32)
    col_embed = np.asarray(col_embed, np.float32)
    w_in = np.asarray(w_in, f8)
    b_in = np.asarray(b_in, f8)
    w_out = np.asarray(w_out, np.float32)
    wq, bq = np.asarray(wq, f8), np.asarray(bq, f8)
    wk, bk = np.asarray(wk, f8), np.asarray(bk, f8)
    wv, bv = np.asarray(wv, f8), np.asarray(bv, f8)

    pos = make_pos(row_embed, col_embed)
    xp = _pack_img(x.reshape(EMB, N_TOK) + pos)
    cp = _pack_img(context.reshape(EMB, N_TOK) + pos)
    # mask: [5120 keys, 5120 queries] -> [128, NW, NJ, WSZ] f16 swizzle
    nm = np.zeros((N_KEY, NQP), np.float16)
    nm[:N_TOK, :N_TOK] = (~pad_mask[0]).T.astype(np.float16)
    nmT = np.ascontiguousarray(
        nm.reshape(NJ, 128, NW, WSZ).transpose(1, 2, 0, 3))

    shared = {"xp": xp, "cp": cp, "nmT": nmT}
    in_maps = []
    for h in range(n_heads):
        sl = slice(h * DEPTH, (h + 1) * DEPTH)
        Aq = wq[sl] @ w_in          # [D, IN_CH]
        cq = wq[sl] @ b_in + bq[sl]
        Ak = wk[sl] @ w_in
        ck = wk[sl] @ b_in + bk[sl]
        Av = wv[sl] @ w_in
        cv = wv[sl] @ b_in + bv[sl]
        f16c = lambda a: np.ascontiguousarray(a.astype(np.float16))
        f32c = lambda a: np.ascontiguousarray(a.astype(np.float32))
        ly = np.zeros((128, EMB), np.float32)
        ly[0:D, :] = w_out[:, sl].T
        ly[64:64 + D, :] = w_out[:, sl].T
        sdn = np.zeros((128, 128), np.float32)
        sdn[D, D] = 1.0
        sdn[64 + D, D] = 1.0
        in_maps.append(dict(
            shared,
            AqT=f16c(np.tile(Aq.T, (1, 4))),
            cq=f32c(np.tile(cq.reshape(DEPTH, 1), (4, 1))),
            AkT=f16c(np.tile(Ak.T, (1, 4))),
            ck=f32c(np.tile(ck.reshape(DEPTH, 1), (4, 1))),
            AvT=f16c(Av.T),
            cvb=f32c(np.broadcast_to(cv, (128, DEPTH))),
            ly=ly,
            sdn=sdn,
        ))
    return in_maps


_CACHE = {}


def kernel(x, context, pad_mask, row_embed, col_embed, w_in, b_in,
           wq, bq, wk, bk, wv, bv, w_out, b_out):
    if "nc" not in _CACHE:
        _CACHE["nc"] = build_nc()
    nc = _CACHE["nc"]
    in_maps = make_in_maps(x, context, pad_mask, row_embed, col_embed,
                           w_in, b_in, wq, bq, wk, bk, wv, bv, w_out)
    res = run_bass_kernel_spmd(nc, in_maps, core_ids=list(range(HEADS)))
    y = np.zeros((EMB, N_TOK), np.float64)
    for c in range(HEADS):
        r = res.results[c]
        y += r["y"].astype(np.float64) / r["dn"].astype(np.float64)
    y = (y + np.asarray(b_out, np.float64)[:, None]).astype(np.float32)
    return y.reshape(EMB, H, W)
